# revision 18
# baseline (speedup 1.0000x reference)
"""MLA (multi-latent attention) prefill kernel for Trainium2, 8 NeuronCores.

Tensor-parallel over heads: each of the 8 cores owns 2 of the 16 heads.
w_q / w_kv_b are column-sharded, w_o row-sharded; the small kv_a latent
projection is replicated. Per-core partial outputs are summed on the host
(the "all-reduce" of the o_proj).

Device-side dataflow (per core, column-major [feature, seq] layouts):
  qT   = wq_mod.T  @ hT     [256, S]  (2 M-blocks: h0[nope|x'], h1[nope|x'])
  rotate_half(x')*sin computed via a +-1 permutation matmul (pmat) on
  x'*sin' (sin' = row-swapped sin table), so no duplicated y weight block.
  kvaT = wkva_mod.T @ hT    [640, S]  (latent 512, then [k-x' | k-y])
  rmsnorm stats: PE ones-matmul over squared latent -> [1,S] -> PE
  broadcast -> sqrt+fast-reciprocal; kv_b runs on the unnormalized latent
  and the scale folds into the k_nope / v output copies.
  scoresT = kT_tile.T @ qT per 128-k-tile -> mask add -> exp -> probsT in
  SBUF (no transposes); k-sums via vector accumulation + one ones-matmul;
  attnT += v_tile.T-ish @ probsT; 1/Z broadcast via PE outer product and
  fused into the attnT PSUM->SBUF evacuation; o = attnT.T @ w_o chunks,
  summed over heads in PSUM, streamed out.

Emission interleave keeps the PE dense: A(c) = projections of seq chunk c,
then attn superblock c-1, then B(c) = stats-dependent kv_b work of chunk c.

Matmul operands are bf16 (full-rate PE, fp32 PSUM accumulation); softmax
statistics, rmsnorm statistics and the final output stay fp32-ish.
"""
import os
import sys
import types
import numpy as np
import ml_dtypes

import concourse.bass as bass
import concourse.mybir as mybir
import concourse.tile as tile
from concourse import bacc, bass_isa, bass_utils, masks

F32 = mybir.dt.float32
BF16 = mybir.dt.bfloat16

S, HID = 2048, 2048
H, NOPE, ROPE, VD, KLR = 16, 64, 64, 128, 512
QD = NOPE + ROPE          # 128
SCALE = QD ** -0.5
EPS = 1e-6
NCORES = 8
HPC = H // NCORES         # heads per core = 2

SC = 512                  # seq chunk for projections
NSC = S // SC             # 4
HC = HID // 128           # 16 hid chunks
NB = S // 512             # 4 superblocks of 512 queries
EXPB = 20.0               # fixed exp bias (overflow headroom)
NEG = -3.0e38


def build_nc():
    nc = bacc.Bacc("TRN2", target_bir_lowering=False, debug=False,
                   num_devices=NCORES)
    dr = {}
    dr["hTp"] = nc.dram_tensor("hTp", [128, NSC, HC, SC], BF16,
                               kind="ExternalInput")
    dr["wq"] = nc.dram_tensor("wq", [128, HC, 256], BF16, kind="ExternalInput")
    dr["wkpe"] = nc.dram_tensor("wkpe", [128, HC, 128], BF16,
                                kind="ExternalInput")
    dr["wklat"] = nc.dram_tensor("wklat", [128, HC, 512], BF16,
                                 kind="ExternalInput")
    dr["wkvb"] = nc.dram_tensor("wkvb", [128, 4, 384], BF16,
                                kind="ExternalInput")
    dr["wo"] = nc.dram_tensor("wo", [128, HPC, HID], BF16,
                              kind="ExternalInput")
    dr["cosd"] = nc.dram_tensor("cosd", [128, S], BF16, kind="ExternalInput")
    dr["sind"] = nc.dram_tensor("sind", [128, S], BF16, kind="ExternalInput")
    dr["sinp"] = nc.dram_tensor("sinp", [128, S], BF16, kind="ExternalInput")
    dr["pmat"] = nc.dram_tensor("pmat", [128, 128], BF16, kind="ExternalInput")
    dr["o"] = nc.dram_tensor("o", [S, HID], BF16, kind="ExternalOutput")

    with tile.TileContext(nc) as tc:
        build_tile_kernel(nc, tc, {k: v.ap() for k, v in dr.items()})
    nc.compile()
    return nc


def build_tile_kernel(nc, tc, d):
    from contextlib import ExitStack
    with ExitStack() as ctx:
        _build_tile_kernel(nc, tc, d, ctx)


def _build_tile_kernel(nc, tc, d, ctx):
    AF = mybir.ActivationFunctionType
    ALU = mybir.AluOpType

    consts = ctx.enter_context(tc.tile_pool(name="consts", bufs=1))
    big = ctx.enter_context(tc.tile_pool(name="big", bufs=1))
    work = ctx.enter_context(tc.tile_pool(name="work", bufs=2))
    htp = ctx.enter_context(tc.tile_pool(name="htp", bufs=2))
    prp = ctx.enter_context(tc.tile_pool(name="prp", bufs=2))
    stat = ctx.enter_context(tc.tile_pool(name="stat", bufs=4))
    outp = ctx.enter_context(tc.tile_pool(name="outp", bufs=2))
    ps = ctx.enter_context(tc.tile_pool(name="ps", bufs=8, space="PSUM"))

    # ---- constants (DMAs ordered + spread by first use) -------------------
    wq_sb = consts.tile([128, HC, 256], BF16)
    wkpe_sb = consts.tile([128, HC, 128], BF16)
    wklat_sb = consts.tile([128, HC, 512], BF16)
    wkvb_sb = consts.tile([128, 4, 384], BF16)
    wo_sb = consts.tile([128, HPC, HID], BF16)
    cos_sb = consts.tile([128, S], BF16)
    sin_sb = consts.tile([128, S], BF16)
    sinp_sb = consts.tile([128, S], BF16)
    pmat_sb = consts.tile([128, 128], BF16)
    ht_tiles = [htp.tile([128, HC, SC], BF16, tag="ht", name=f"ht{c}")
                for c in range(2)]

    # critical first wave only: wq + ht0 + wkpe get full DMA bandwidth.
    # Everything else is issued later in emission order, just before use.
    nc.scalar.dma_start(out=wq_sb[:], in_=d["wq"])
    nc.sync.dma_start(out=ht_tiles[0][:, 0:HC // 2, :],
                      in_=d["hTp"][:, 0, 0:HC // 2, :])
    nc.scalar.dma_start(out=ht_tiles[0][:, HC // 2:HC, :],
                        in_=d["hTp"][:, 0, HC // 2:HC, :])
    nc.sync.dma_start(out=wkpe_sb[:], in_=d["wkpe"])
    nc.gpsimd.dma_start(out=pmat_sb[:], in_=d["pmat"])
    nc.gpsimd.dma_start(out=sin_sb[:], in_=d["sind"])

    ones_sb = consts.tile([128, 1], BF16)
    nc.vector.memset(ones_sb[:], 1.0)
    onesrow_sb = consts.tile([1, 128], BF16)
    nc.vector.memset(onesrow_sb[:], 1.0)
    one_f32 = consts.tile([1, 1], F32)
    nc.vector.memset(one_f32[:], 1.0)
    eps_sb2 = consts.tile([128, 1], F32)
    nc.vector.memset(eps_sb2[:], EPS)
    nexpb_sb = consts.tile([128, 1], F32)
    nc.vector.memset(nexpb_sb[:], -EXPB)
    # causal strip for scoresT: trimask[p, t] = 0 if t >= p + 384 else NEG
    trimask = consts.tile([128, 896], F32)
    nc.gpsimd.memset(trimask[:], 0.0)
    nc.gpsimd.affine_select(out=trimask[:], in_=trimask[:],
                            compare_op=ALU.is_ge, fill=NEG, base=-384,
                            pattern=[[1, 896]], channel_multiplier=-1)

    # ---- persistent activations -----------------------------------------
    qT = [big.tile([128, S], BF16, tag=f"qT{h}", name=f"qT{h}")
          for h in range(HPC)]
    kT = [big.tile([128, S], BF16, tag=f"kT{h}", name=f"kT{h}")
          for h in range(HPC)]
    v_sb = big.tile([128, S // 128, HPC * VD], BF16, tag="v")

    # per-chunk carried tiles (written in A(c), consumed in B(c))
    state = {}  # keyed by chunk

    # =====================================================================
    # A(c): projection waves + rope for seq chunk c (512 cols).
    # =====================================================================
    def proj_a(c):
        cs = slice(c * SC, (c + 1) * SC)
        ht_r = ht_tiles[c % 2]

        # ---- wave 1: q blocks, then shared k_pe block ----
        pq = [ps.tile([128, SC], F32, tag="ps", name=f"pq{i}")
              for i in range(HPC)]
        pkpe = ps.tile([128, SC], F32, tag="ps", name="pkpe")
        for k in range(HC):
            for m in range(HPC):
                nc.tensor.matmul(pq[m][:], wq_sb[:, k, m * 128:(m + 1) * 128],
                                 ht_r[:, k, :], start=(k == 0),
                                 stop=(k == HC - 1))
        if c == 0:
            # deferred loads: needed by rope epilogue / wave2, issued once
            nc.scalar.dma_start(out=cos_sb[:], in_=d["cosd"])
            nc.sync.dma_start(out=sinp_sb[:], in_=d["sinp"])
            nc.sync.dma_start(out=wklat_sb[:], in_=d["wklat"])
        for k in range(HC):
            nc.tensor.matmul(pkpe[:], wkpe_sb[:, k, 0:128], ht_r[:, k, :],
                             start=(k == 0), stop=(k == HC - 1))

        # q rope: y*sin via permutation matmul on x'*sin'
        s2 = work.tile([128, SC], BF16, tag="s2")
        for h in range(HPC):
            nc.vector.tensor_tensor(s2[64 * h:64 * (h + 1), :],
                                    pq[h][64:128, :],
                                    sinp_sb[64 * h:64 * (h + 1), cs], ALU.mult)
        py = ps.tile([128, SC], F32, tag="ps", name="py")
        nc.tensor.matmul(py[:], pmat_sb[:], s2[:], start=True, stop=True)
        for h in range(HPC):
            nc.vector.tensor_copy(qT[h][0:64, cs], pq[h][0:64, :])
            nc.vector.tensor_tensor(qT[h][64:128, cs], pq[h][64:128, :],
                                    cos_sb[64:128, cs], ALU.mult)
            nc.vector.tensor_tensor(qT[h][64:128, cs], qT[h][64:128, cs],
                                    py[64 * h:64 * (h + 1), :], ALU.add)
        # k_pe rope (shared across heads; [x'|y] block like baseline)
        nc.vector.tensor_tensor(kT[0][64:128, cs], pkpe[0:64, :],
                                cos_sb[0:64, cs], ALU.mult)
        t2 = work.tile([128, SC], F32, tag="t2")
        nc.vector.tensor_tensor(t2[64:128, :], pkpe[64:128, :],
                                sin_sb[64:128, cs], ALU.mult)
        nc.vector.tensor_tensor(kT[0][64:128, cs], kT[0][64:128, cs],
                                t2[64:128, :], ALU.add)
        nc.vector.tensor_copy(kT[1][64:128, cs], kT[0][64:128, cs])

        # ---- wave 2: latent blocks ----
        plat = [ps.tile([128, SC], F32, tag="ps", name=f"plat{i}")
                for i in range(4)]
        for k in range(HC):
            for m in range(4):
                nc.tensor.matmul(plat[m][:],
                                 wklat_sb[:, k, m * 128:(m + 1) * 128],
                                 ht_r[:, k, :], start=(k == 0),
                                 stop=(k == HC - 1))
        ckv = work.tile([128, 4, SC], BF16, tag="ckv", bufs=2)
        for m in range(4):
            nc.scalar.copy(ckv[:, m, :], plat[m][:])
        sq = work.tile([128, 4, SC], BF16, tag="sq", bufs=2)
        nc.vector.tensor_tensor(sq[:], ckv[:], ckv[:], ALU.mult)
        state[c] = (ckv, sq)
        if c == 0:
            nc.scalar.dma_start(out=wkvb_sb[:], in_=d["wkvb"])
            nc.sync.dma_start(out=ht_tiles[1][:], in_=d["hTp"][:, 1, :, :])
        elif c == 1:
            nc.scalar.dma_start(out=wo_sb[:], in_=d["wo"])
        # prefetch ht chunk c+2 into the buffer wave2 just finished reading
        if c + 2 < NSC:
            nc.sync.dma_start(out=ht_tiles[(c + 2) % 2][:],
                              in_=d["hTp"][:, c + 2, :, :])

    # =====================================================================
    # B(c): rmsnorm stats (PE path) + kv_b, emitted after attn(c-1).
    # =====================================================================
    def proj_b(c):
        cs = slice(c * SC, (c + 1) * SC)
        ckv, sq = state.pop(c)

        # sum of squares over latent dim -> [1, SC] via ones-matmul
        pssq = ps.tile([1, SC], F32, tag="ps", name="pssq")
        for m in range(4):
            nc.tensor.matmul(pssq[:], ones_sb[:], sq[:, m, :],
                             start=(m == 0), stop=(m == 3))
        srow = work.tile([1, SC], BF16, tag="srow")
        nc.vector.tensor_copy(srow[:], pssq[:])

        # k_nope (column-major) on unnormalized latent
        pnope = ps.tile([128, SC], F32, tag="ps", name="pnope")
        for kk in range(4):
            nc.tensor.matmul(pnope[:], wkvb_sb[:, kk, 0:128], ckv[:, kk, :],
                             start=(kk == 0), stop=(kk == 3))

        # broadcast sumsq to all partitions: ones_col x srow
        pbc = ps.tile([128, SC], F32, tag="ps", name="pbc")
        nc.tensor.matmul(pbc[:], onesrow_sb[:], srow[:], start=True, stop=True)
        sbc = work.tile([128, SC], F32, tag="sbc")
        nc.scalar.activation(sbc[:], pbc[:], AF.Sqrt, bias=eps_sb2[:],
                             scale=1.0 / KLR)
        nc.vector.reciprocal_approx_fast(sbc[:], sbc[:])

        # v rows on unnormalized latent
        pvs = []
        for t in range(4):
            pv = ps.tile([128, HPC * VD], F32, tag="ps", name=f"pv{t}")
            for kk in range(4):
                nc.tensor.matmul(pv[:], ckv[:, kk, t * 128:(t + 1) * 128],
                                 wkvb_sb[:, kk, 128:384],
                                 start=(kk == 0), stop=(kk == 3))
            pvs.append(pv)

        # rms scale as a per-seq column vector for the row-major v scaling
        pcol = ps.tile([128, 4], F32, tag="ps", name="pcol")
        for t in range(4):
            nc.tensor.transpose(pcol[:, t:t + 1],
                                sbc[0:1, t * 128:(t + 1) * 128],
                                one_f32[0:1, 0:1])
        scol = stat.tile([128, 4], F32, tag="scol")
        nc.vector.tensor_copy(scol[:], pcol[:])

        # scale fused into the output copies
        nc.vector.tensor_tensor(kT[0][0:64, cs], pnope[0:64, :],
                                sbc[0:64, 0:SC], ALU.mult)
        nc.vector.tensor_tensor(kT[1][0:64, cs], pnope[64:128, :],
                                sbc[64:128, 0:SC], ALU.mult)
        for t in range(4):
            nc.vector.tensor_scalar_mul(v_sb[:, 4 * c + t, :], pvs[t][:],
                                        scol[:, t:t + 1])

    # =====================================================================
    # attn superblock B: 512 queries vs k tiles 0..4(B+1)-1, scoresT layout.
    # =====================================================================
    def attn_block(B):
        nkt = 4 * (B + 1)
        qs = slice(B * 512, (B + 1) * 512)
        pr = [None, None]
        acc = [None, None]
        # scores + exp (subrange on diagonal tiles), per head
        for h in range(HPC):
            pr[h] = prp.tile([128, 16, 512], BF16, tag="probs",
                             name=f"pr{h}")
            for kt in range(nkt):
                j = kt - 4 * B
                lo = 128 * j if j > 0 else 0   # first valid q col
                psc = ps.tile([128, 512], F32, tag="ps", name="psc")
                nc.tensor.matmul(psc[:, lo:512],
                                 kT[h][:, kt * 128:(kt + 1) * 128],
                                 qT[h][:, B * 512 + lo:(B + 1) * 512],
                                 start=True, stop=True)
                if j >= 0:          # triangle mask on the 128-col diagonal
                    nc.vector.tensor_tensor(psc[:, lo:lo + 128],
                                            psc[:, lo:lo + 128],
                                            trimask[:, 384:512], ALU.add)
                nc.scalar.activation(pr[h][:, kt, lo:512], psc[:, lo:512],
                                     bias=nexpb_sb[:], scale=1.0,
                                     func=AF.Exp)
                if j > 0:
                    nc.vector.memset(pr[h][:, kt, 0:lo], 0.0)
                if kt == 0:
                    acc[h] = work.tile([128, 512], BF16, tag=f"acc{h}",
                                       name=f"acc{h}")
                    nc.vector.tensor_copy(acc[h][:], pr[h][:, 0, :])
                else:
                    nc.vector.tensor_tensor(acc[h][:], acc[h][:],
                                            pr[h][:, kt, :], ALU.add)

        # PV + Z matmuls (interleaved so the Z chain latency hides)
        pa = [None, None]
        pz = [None, None]
        zr = [None, None]
        for h in range(HPC):
            pa[h] = ps.tile([128, 512], F32, tag="ps", name=f"pa{h}")
            for kt in range(nkt):
                nc.tensor.matmul(pa[h][:], v_sb[:, kt, h * VD:(h + 1) * VD],
                                 pr[h][:, kt, :], start=(kt == 0),
                                 stop=(kt == nkt - 1))
            pz[h] = ps.tile([1, 512], F32, tag="ps", name=f"pz{h}")
            nc.tensor.matmul(pz[h][:], ones_sb[:], acc[h][:],
                             start=True, stop=True)
            zr[h] = work.tile([1, 512], BF16, tag=f"zr{h}", name=f"zr{h}")
            nc.scalar.copy(zr[h][:], pz[h][:])
        attnT = [None, None]
        for h in range(HPC):
            prb = ps.tile([128, 512], F32, tag="ps", name=f"prb{h}")
            nc.tensor.matmul(prb[:], onesrow_sb[:], zr[h][:],
                             start=True, stop=True)
            rbc = work.tile([128, 512], F32, tag=f"rbc{h}")
            nc.vector.tensor_copy(rbc[:], prb[:])
            nc.vector.reciprocal_approx_fast(rbc[:], rbc[:])
            at = work.tile([128, 512], BF16, tag=f"attnT{h}")
            nc.vector.tensor_tensor(at[:], pa[h][:], rbc[:], ALU.mult)
            attnT[h] = at

        # ---- o_proj for the four seq tiles of this superblock ----
        for t in range(4):
            ot = outp.tile([128, 4, 512], BF16, tag="ot")
            for n in range(4):
                po = ps.tile([128, 512], F32, tag="ps", name="po")
                for h in range(HPC):
                    nc.tensor.matmul(po[:],
                                     attnT[h][:, t * 128:(t + 1) * 128],
                                     wo_sb[:, h, n * 512:(n + 1) * 512],
                                     start=(h == 0), stop=(h == HPC - 1))
                if n % 2 == 0:
                    nc.vector.tensor_copy(ot[:, n, :], po[:])
                else:
                    nc.scalar.copy(ot[:, n, :], po[:])
            rows = slice((4 * B + t) * 128, (4 * B + t + 1) * 128)
            if B == NSC - 1 and t == 3:
                nc.sync.dma_start(out=d["o"][rows, 0:1024], in_=ot[:, 0:2, :])
                nc.scalar.dma_start(out=d["o"][rows, 1024:2048],
                                    in_=ot[:, 2:4, :])
            else:
                eng = nc.sync if t % 2 == 0 else nc.scalar
                eng.dma_start(out=d["o"][rows, :], in_=ot[:])

    proj_a(0)
    proj_a(1)
    proj_b(0)
    attn_block(0)
    proj_a(2)
    proj_b(1)
    attn_block(1)
    proj_a(3)
    proj_b(2)
    attn_block(2)
    proj_b(3)
    attn_block(3)


# =========================================================================
# host side
# =========================================================================
_perm1 = np.concatenate([np.arange(0, ROPE, 2), np.arange(1, ROPE, 2)])
_perm2 = np.concatenate([np.arange(1, ROPE, 2), np.arange(0, ROPE, 2)])
_sgn2 = np.concatenate([-np.ones(ROPE // 2), np.ones(ROPE // 2)]).astype(np.float32)
_swap = np.concatenate([np.arange(32, 64), np.arange(0, 32)])


def _host_prep(inputs):
    bf = ml_dtypes.bfloat16
    hidden = np.ascontiguousarray(np.asarray(inputs["hidden_states"],
                                             dtype=np.float32)[0])
    cos = np.asarray(inputs["cos"], dtype=np.float32)[0]
    sin = np.asarray(inputs["sin"], dtype=np.float32)[0]
    w_q = np.asarray(inputs["w_q"], dtype=np.float32)
    w_kv_a = np.asarray(inputs["w_kv_a"], dtype=np.float32)
    ln_w = np.asarray(inputs["kv_a_ln_w"], dtype=np.float32)
    w_kv_b = np.asarray(inputs["w_kv_b"], dtype=np.float32)
    w_o = np.asarray(inputs["w_o"], dtype=np.float32)

    # hTp[p, c, k, m] = hidden[c*512+m, k*128+p]
    hTp = np.ascontiguousarray(
        hidden.reshape(NSC, SC, HC, 128).transpose(3, 0, 2, 1)).astype(bf)
    cosT = cos.T                       # [64, S]
    sinT = sin.T
    cosd = np.ascontiguousarray(np.concatenate([cosT, cosT], axis=0)).astype(bf)
    sind = np.ascontiguousarray(np.concatenate([sinT, sinT], axis=0)).astype(bf)
    sinp = np.ascontiguousarray(np.concatenate([sinT[_swap], sinT[_swap]],
                                               axis=0)).astype(bf)
    # pmat[j, i] = sgn_i if j == swap(i), block-diagonal over two 64-blocks
    pmat = np.zeros((128, 128), dtype=np.float32)
    for blk in range(2):
        for i in range(64):
            sgn = -1.0 if i < 32 else 1.0
            pmat[blk * 64 + _swap[i], blk * 64 + i] = sgn
    pmat = pmat.astype(bf)

    kpe_cols = w_kv_a[:, KLR:]
    wkpe_mod = np.concatenate(
        [kpe_cols[:, _perm1], kpe_cols[:, _perm2] * _sgn2[None, :]], axis=1)
    wkpe_p = np.ascontiguousarray(
        wkpe_mod.reshape(HC, 128, 128).transpose(1, 0, 2)).astype(bf)
    wklat_p = np.ascontiguousarray(
        w_kv_a[:, :KLR].reshape(HC, 128, 512).transpose(1, 0, 2)).astype(bf)
    wkvb_all = w_kv_b * ln_w[:, None]

    in_maps = []
    for cidx in range(NCORES):
        heads = [HPC * cidx + i for i in range(HPC)]
        blocks = []
        for h in heads:
            wq_h = w_q[:, h * QD:(h + 1) * QD]
            blocks.append(np.concatenate(
                [wq_h[:, :NOPE], wq_h[:, NOPE:][:, _perm1]], axis=1))
        wq_mod = np.concatenate(blocks, axis=1) * SCALE     # [2048, 256]
        wq_p = np.ascontiguousarray(
            wq_mod.reshape(HC, 128, 256).transpose(1, 0, 2)).astype(bf)

        nope_b = [wkvb_all[:, h * (NOPE + VD):h * (NOPE + VD) + NOPE]
                  for h in heads]
        v_b = [wkvb_all[:, h * (NOPE + VD) + NOPE:(h + 1) * (NOPE + VD)]
               for h in heads]
        wkvb_mod = np.concatenate(nope_b + v_b, axis=1)     # [512, 384]
        wkvb_p = np.ascontiguousarray(
            wkvb_mod.reshape(4, 128, 384).transpose(1, 0, 2)).astype(bf)

        wo_mod = w_o[heads[0] * VD:(heads[-1] + 1) * VD, :]  # [256, 2048]
        wo_p = np.ascontiguousarray(
            wo_mod.reshape(HPC, 128, HID).transpose(1, 0, 2)).astype(bf)

        in_maps.append({"hTp": hTp, "wq": wq_p, "wkpe": wkpe_p,
                        "wklat": wklat_p, "wkvb": wkvb_p, "wo": wo_p,
                        "cosd": cosd, "sind": sind, "sinp": sinp,
                        "pmat": pmat})
    return in_maps


def _install_ntff_hook():
    """Make trace=True work under axon (antenv.axon_hooks is absent in this
    image; back it with trn_agent_boot's ctypes hook)."""
    try:
        import antenv
        if "antenv.axon_hooks" in sys.modules:
            return
        from trn_agent_boot.trn_boot import _ntff_profile_via_ctypes
        hook = _ntff_profile_via_ctypes("/opt/axon/libaxon_pjrt.so")
        mod = types.ModuleType("antenv.axon_hooks")
        mod.get_axon_ntff_profile_hook = lambda: hook
        mod.set_axon_ntff_profile_hook = lambda h: None
        sys.modules["antenv.axon_hooks"] = mod
        antenv.axon_hooks = mod
    except Exception:
        pass


_nc_cache = None
last_results = None


def kernel(**inputs):
    global _nc_cache, last_results
    _install_ntff_hook()
    if _nc_cache is None:
        _nc_cache = build_nc()
    in_maps = _host_prep(inputs)
    trace = bool(os.environ.get("BASS_TRACE"))
    res = bass_utils.run_bass_kernel_spmd(
        _nc_cache, in_maps, core_ids=list(range(NCORES)), trace=trace)
    last_results = res
    total = res.results[0]["o"].astype(np.float32)
    for c in range(1, NCORES):
        total = total + res.results[c]["o"]
    return total.reshape(1, S, HID)
